# revision 19
# baseline (speedup 1.0000x reference)
"""Trainium2 Bass kernel for nn_CausalSelfAttention (modded-nanogpt quantized attention).

Sharding: 8 cores = 2 batches x 4 head-groups (2 heads each).
Each core computes QKV for its 2 heads from the full x[b], runs causal
attention + gating for those heads, and produces a partial output
projection (its 256 feature columns of w_o); the host sums the 4 partials
per batch.

Host-side prep (exact, bit-mirrors the reference):
 - x int8 fake-quant computed in numpy and shipped PRE-TRANSPOSED (xqT),
   removing the whole on-device x-quant pipeline and its PE transposes.
 - qkv / o weights ternary-quantized on host; only the {-1,0,1} CODES are
   shipped (exact in any dtype); global scales fold into per-head alpha,
   the v-mix scalar and the gate scalar.

Device-side structure per core:
 - All matmul operands are float32r: full PE speed (1 cycle/row when the
   moving free dim >= 256) and EXACT for integer codes; ~1.5e-4 rel err
   on value matmuls (scores / PV / proj) - well within tolerance.
 - QKV natural [t,768] from xqT-codes @ w-codes (exact integer arithmetic).
 - q/k chains processed JOINTLY as [128, 4, 128] (q0 q1 k0 k1): rmsnorm
   alpha (rms folded with the ternary scale), rotary, int8 fake-quant with
   per-head scalars applied through stride-0 broadcast APs.
 - scores transposed S_T[tk,tq] = kT.T @ qT; exp on ACT (scale fused),
   causal diag mask via affine_select; y = E.T @ [v|1] per 512-wide tq
   strip; normalize+gate per token; PE-transpose y into yT; partial
   output projection from yT against w_o codes.
"""

import numpy as np

B, T, DIM, H, HD = 2, 2048, 1024, 8, 128
ATTN_SCALE = 0.12
F32_EPS = float(np.finfo(np.float32).eps)
MAGIC = float(np.float32(1.5 * 2 ** 23))  # RNE round for |x| < 2^22
NT = T // 128          # 16 t-tiles
ND = DIM // 128        # 8 d-tiles
HLOC = 2               # heads per core
ELOC = HLOC * HD       # 256 local features
NSTRIP = T // 512      # 4 tq strips per head

_CACHE = {}


def _build():
    import concourse.bass as bass
    import concourse.mybir as mybir
    import concourse.tile as tile
    from concourse import bacc
    from concourse.masks import make_identity
    from contextlib import ExitStack

    f32 = mybir.dt.float32
    f32r = mybir.dt.float32r
    A = mybir.AluOpType
    AF = mybir.ActivationFunctionType
    X = mybir.AxisListType.X

    nc = bacc.Bacc(trn_type="TRN2")

    # extra activation-bias constants (Bass pre-registers only 0.0/1.0)
    for _v in (MAGIC, F32_EPS):
        _t = nc.alloc_sbuf_tensor(f"const-float32-{_v}", [128, 1], f32)
        nc.gpsimd.memset(_t.ap(), _v)
        nc.const_aps.aps[(f32, _v)] = _t.ap()
    nc.all_engine_barrier()

    xqT_d = nc.dram_tensor("xqT", [DIM, T], f32r, kind="ExternalInput")
    tau_d = nc.dram_tensor("tau", [DIM, 3 * ELOC], f32r, kind="ExternalInput")
    tauo_d = nc.dram_tensor("tauo", [ELOC, DIM], f32r, kind="ExternalInput")
    c2_d = nc.dram_tensor("c2", [T, HD], f32, kind="ExternalInput")
    s2_d = nc.dram_tensor("s2", [T, HD], f32, kind="ExternalInput")
    vel_d = nc.dram_tensor("vel", [T, ELOC], f32, kind="ExternalInput")
    gw_d = nc.dram_tensor("gw", [12, HLOC], f32r, kind="ExternalInput")
    # scal cols: 0:4 s4=[sq,sq,sk,sk], 4:8 sq2=[sq^2,..,sk^2], 8 lam0*sv, 9 s_o
    scal_d = nc.dram_tensor("scal", [128, 10], f32, kind="ExternalInput")
    outp = nc.dram_tensor("outp", [T, DIM], f32, kind="ExternalOutput")
    import os
    DBG = int(os.environ.get("KDBG", "0"))
    if DBG:
        dbg_nat = nc.dram_tensor("dbg_nat", [T, 768], f32, kind="ExternalOutput")
        dbg_qq = nc.dram_tensor("dbg_qq", [T, 512], f32, kind="ExternalOutput")
        dbg_g = nc.dram_tensor("dbg_g", [T, HLOC], f32, kind="ExternalOutput")
        dbg_va = nc.dram_tensor("dbg_va", [T, ELOC], f32, kind="ExternalOutput")
        dbg_yt = nc.dram_tensor("dbg_yt", [T, ELOC], f32, kind="ExternalOutput")

    def bc(src, like):
        b, _ = bass.broadcast_tensor_aps(src, like)
        return b

    with tile.TileContext(nc) as tc, ExitStack() as ctx:
        singles = ctx.enter_context(tc.tile_pool(name="singles", bufs=1))
        inpool = ctx.enter_context(tc.tile_pool(name="inpool", bufs=2))
        qkpool = ctx.enter_context(tc.tile_pool(name="qkpool", bufs=2))
        scl = ctx.enter_context(tc.tile_pool(name="scl", bufs=4))
        epool = ctx.enter_context(tc.tile_pool(name="epool", bufs=2 if DBG else 3))
        ypool = ctx.enter_context(tc.tile_pool(name="ypool", bufs=2))
        opool = ctx.enter_context(tc.tile_pool(name="opool", bufs=2))
        dpool = ctx.enter_context(tc.tile_pool(name="dpool", bufs=1)) if DBG else None
        psQK = ctx.enter_context(tc.tile_pool(name="psQK", bufs=2, space="PSUM"))
        psY = ctx.enter_context(tc.tile_pool(name="psY", bufs=4, space="PSUM"))

        def ts(out, in0, s1, s2=None, op0=A.mult, op1=None, eng=None):
            e = eng if eng is not None else nc.vector
            kw = {}
            if op1 is not None:
                kw["op1"] = op1
            e.tensor_scalar(out=out, in0=in0, scalar1=s1, scalar2=s2, op0=op0, **kw)

        # ---------------- constants / persistent inputs ----------------
        ident_f = singles.tile([128, 128], f32)
        make_identity(nc, ident_f)
        ident = singles.tile([128, 128], f32r)
        nc.vector.tensor_copy(out=ident, in_=ident_f)

        scal_sb = singles.tile([128, 10], f32)
        nc.sync.dma_start(out=scal_sb, in_=scal_d[:, :])
        gw_sb = singles.tile([12, HLOC], f32r)
        nc.sync.dma_start(out=gw_sb, in_=gw_d[:, :])

        tau = singles.tile([128, ND, 3 * ELOC], f32r)
        nc.sync.dma_start(out=tau, in_=tau_d.rearrange("(n p) e -> p n e", p=128))
        tauo = singles.tile([128, HLOC, DIM], f32r)
        nc.sync.dma_start(out=tauo, in_=tauo_d.rearrange("(h p) d -> p h d", p=128))
        xqT = singles.tile([128, ND, T], f32r)
        nc.sync.dma_start(out=xqT, in_=xqT_d.rearrange("(n p) t -> p n t", p=128))

        # persistent activations
        qkT = singles.tile([128, 4, T], f32r)          # q0 q1 k0 k1, [hd, t]
        vaug = singles.tile([128, HLOC, NT, HD + 2], f32r)
        ones_f = singles.tile([128, NT, 2], f32)
        nc.gpsimd.memset(ones_f[:, :, 0:1], 1.0)
        nc.gpsimd.memset(ones_f[:, :, 1:2], 0.0)
        for _h in range(HLOC):
            nc.vector.tensor_copy(out=vaug[:, _h, :, HD:HD + 2], in_=ones_f)
        gate_so = singles.tile([128, NT, HLOC], f32)   # sigmoid(gate)*s_o
        yT = singles.tile([128, HLOC, T], f32r)        # [hd, t] gated y

        # ======== attention strip + projection emitters (interleaved) ========
        def emit_strip(h, J):
            Jsl = slice(J * 512, (J + 1) * 512)
            yu0 = psY.tile([128, HD + 2], f32, tag="yu")
            yu1 = psY.tile([128, HD + 2], f32, tag="yu")
            yu2 = psY.tile([128, HD + 2], f32, tag="yu")
            yu3 = psY.tile([128, HD + 2], f32, tag="yu")
            yu = [yu0, yu1, yu2, yu3]
            for i in range(4 * J + 4):
                st = psQK.tile([128, 4, 128], f32, tag="qk")
                stf = st.rearrange("p a b -> p (a b)")
                nc.tensor.matmul(stf, qkT[:, 2 + h, i * 128:(i + 1) * 128],
                                 qkT[:, h, Jsl], start=True, stop=True)
                lo = max(0, 128 * (i - 4 * J))
                E = epool.tile([128, 512], f32r, tag="E")
                nc.scalar.activation(E[:, lo:512], stf[:, lo:512], AF.Exp,
                                     scale=ATTN_SCALE)
                if i >= 4 * J:
                    dl = 128 * (i - 4 * J)
                    nc.gpsimd.affine_select(
                        out=E[:, dl:dl + 128], in_=E[:, dl:dl + 128],
                        compare_op=A.is_ge, fill=0.0, base=0,
                        pattern=[[1, 128]], channel_multiplier=-1)
                for j in range(max(4 * J, i), 4 * J + 4):
                    jj = j - 4 * J
                    nc.tensor.matmul(yu[jj], E[:, jj * 128:(jj + 1) * 128],
                                     vaug[:, h, i, :],
                                     start=(i == 0), stop=(i == j),
                                     skip_group_check=True)
            for jj in range(4):
                j = 4 * J + jj
                den = scl.tile([128, 1], f32, tag="den")
                nc.vector.reciprocal(out=den, in_=yu[jj][:, HD:HD + 1])
                gam = scl.tile([128, 1], f32, tag="gam")
                nc.vector.tensor_tensor(out=gam, in0=den,
                                        in1=gate_so[:, j, h:h + 1], op=A.mult)
                ynat = ypool.tile([128, HD], f32r, tag="ynat")
                ts(ynat, yu[jj][:, 0:HD], gam)
                psy = psY.tile([128, 4, 128], f32r, tag="yu")
                nc.tensor.transpose(psy[:, 0, :], ynat, ident)
                nc.vector.tensor_copy(out=yT[:, h, j * 128:(j + 1) * 128],
                                       in_=psy[:, 0, :])

        def emit_proj(J):
            for jj in range(4):
                j = 4 * J + jj
                jsl = slice(j * 128, (j + 1) * 128)
                osb = opool.tile([128, DIM], f32, tag="osb")
                for half in range(2):
                    op_ = psQK.tile([128, 512], f32, tag="qk")
                    for h in range(HLOC):
                        nc.tensor.matmul(op_, yT[:, h, jsl],
                                         tauo[:, h, half * 512:(half + 1) * 512],
                                         start=(h == 0), stop=(h == HLOC - 1))
                    nc.vector.tensor_copy(out=osb[:, half * 512:(half + 1) * 512],
                                          in_=op_)
                nc.sync.dma_start(out=outp[jsl, :], in_=osb)
                if DBG:
                    pd = psY.tile([128, 4, 128], f32r, tag="yu")
                    for h in range(HLOC):
                        nc.tensor.transpose(pd[:, h, :], yT[:, h, jsl], ident)
                    ydf = dpool.tile([128, 256], f32, tag="d256")
                    nc.vector.tensor_copy(out=ydf, in_=pd[:, 0:2, :])
                    nc.sync.dma_start(out=dbg_yt[jsl, :], in_=ydf)

        # ======== phase 1: per t-tile QKV -> q/k chain -> v mix ========
        for i in range(NT):
            tsl = slice(i * 128, (i + 1) * 128)
            c2t = inpool.tile([128, 1, HD], f32, tag="c2t")
            s2t = inpool.tile([128, 1, HD], f32, tag="s2t")
            velt = inpool.tile([128, HLOC, HD], f32, tag="velt")
            nc.sync.dma_start(out=c2t, in_=c2_d[tsl, :].rearrange("p (o e) -> p o e", o=1))
            nc.sync.dma_start(out=s2t, in_=s2_d[tsl, :].rearrange("p (o e) -> p o e", o=1))
            nc.sync.dma_start(out=velt, in_=vel_d[tsl, :].rearrange("p (h e) -> p h e", h=HLOC))

            # QKV matmuls (codes x codes: exact)
            qk_ps = psQK.tile([128, 2 * ELOC], f32, tag="qk")
            v_ps = psQK.tile([128, ELOC], f32, tag="v")
            for d in range(ND):
                nc.tensor.matmul(qk_ps, xqT[:, d, tsl], tau[:, d, 0:2 * ELOC],
                                 start=(d == 0), stop=(d == ND - 1))
                nc.tensor.matmul(v_ps, xqT[:, d, tsl], tau[:, d, 2 * ELOC:3 * ELOC],
                                 start=(d == 0), stop=(d == ND - 1))

            # gate logits from quantized-x rows 0..11
            g_ps = psY.tile([128, HLOC], f32, tag="yu")
            nc.tensor.matmul(g_ps, xqT[0:12, 0, tsl], gw_sb, start=True, stop=True)
            nc.scalar.activation(gate_so[:, i, :], g_ps, AF.Sigmoid)
            ts(gate_so[:, i, :], gate_so[:, i, :], scal_sb[:, 9:10])

            # v mix: vaug = (lam0*sv)*v_hat + vel  (both heads at once)
            v3 = v_ps.rearrange("p (h e) -> p h e", h=HLOC)
            nc.vector.scalar_tensor_tensor(
                out=vaug[:, :, i, 0:HD], in0=v3, scalar=scal_sb[:, 8:9],
                in1=velt, op0=A.mult, op1=A.add)

            # ---- joint q/k chain on [128, 4, 128] ----
            nat = qkpool.tile([128, 4, HD], f32r, tag="nat")
            nc.scalar.activation(nat, qk_ps, AF.Identity)

            t2 = qkpool.tile([128, 4, HD], f32, tag="t2")
            nc.vector.tensor_tensor(out=t2, in0=nat, in1=nat, op=A.mult)
            ssq = scl.tile([128, 4], f32, tag="ssq")
            nc.vector.tensor_reduce(
                out=ssq.rearrange("p (h o) -> p h o", o=1), in_=t2, axis=X, op=A.add)
            # alpha = s / sqrt(ssq*s^2/HD + eps)   [128,4]
            al4 = scl.tile([128, 4], f32, tag="al4")
            nc.vector.scalar_tensor_tensor(out=al4, in0=ssq, scalar=1.0 / HD,
                                           in1=scal_sb[:, 4:8], op0=A.mult, op1=A.mult)
            nc.scalar.activation(al4, al4, AF.Sqrt, bias=F32_EPS)
            nc.vector.reciprocal(out=al4, in_=al4)
            nc.vector.tensor_tensor(out=al4, in0=al4, in1=scal_sb[:, 0:4], op=A.mult)

            # rotary: rot = nat*c2 + shuf(nat)*s2  (c2/s2 broadcast over 4 chunks)
            rot = qkpool.tile([128, 4, HD], f32, tag="rot")
            nc.vector.tensor_tensor(out=rot, in0=nat, in1=bc(c2t, rot), op=A.mult)
            nc.vector.tensor_tensor(out=t2[:, :, 0:64], in0=nat[:, :, 64:128],
                                    in1=bc(s2t[:, :, 0:64], t2[:, :, 0:64]), op=A.mult)
            nc.vector.tensor_tensor(out=t2[:, :, 64:128], in0=nat[:, :, 0:64],
                                    in1=bc(s2t[:, :, 64:128], t2[:, :, 64:128]), op=A.mult)
            nc.vector.tensor_tensor(out=rot, in0=rot, in1=t2, op=A.add)

            # per-head min/max and quant scales
            mx4 = scl.tile([128, 4], f32, tag="mx4")
            mn4 = scl.tile([128, 4], f32, tag="mn4")
            nc.vector.tensor_reduce(
                out=mx4.rearrange("p (h o) -> p h o", o=1), in_=rot, axis=X, op=A.max)
            nc.vector.tensor_reduce(
                out=mn4.rearrange("p (h o) -> p h o", o=1), in_=rot, axis=X, op=A.min)
            xpm = scl.tile([128, 4], f32, tag="xpm")
            xnm = scl.tile([128, 4], f32, tag="xnm")
            ts(xpm, mx4, 1e-5, None, A.max)
            ts(xnm, mn4, -1e-5, None, A.min)
            mp4 = scl.tile([128, 4], f32, tag="mp4")
            mnn4 = scl.tile([128, 4], f32, tag="mnn4")
            nc.vector.reciprocal(out=mp4, in_=xpm)
            nc.vector.reciprocal(out=mnn4, in_=xnm)
            ts(mp4, mp4, 127.0)
            ts(mnn4, mnn4, 127.0)
            spal = scl.tile([128, 4], f32, tag="spal")
            snal = scl.tile([128, 4], f32, tag="snal")
            nc.vector.scalar_tensor_tensor(out=spal, in0=xpm, scalar=1.0 / 127.0,
                                           in1=al4, op0=A.mult, op1=A.mult)
            nc.vector.scalar_tensor_tensor(out=snal, in0=xnm, scalar=1.0 / 127.0,
                                           in1=al4, op0=A.mult, op1=A.mult)

            def b4(t_):  # [128,4] -> broadcast [128,4,128]
                return bc(t_.rearrange("p (h o) -> p h o", o=1), rot)

            # two-branch int8 fake-quant (RNE via magic constant)
            zp = qkpool.tile([128, 4, HD], f32, tag="zp")
            zn = qkpool.tile([128, 4, HD], f32, tag="zn")
            nc.vector.scalar_tensor_tensor(out=zp, in0=rot, scalar=0.0, in1=b4(mp4),
                                           op0=A.max, op1=A.mult)
            nc.scalar.activation(zp, zp, AF.Identity, bias=MAGIC)
            nc.vector.scalar_tensor_tensor(out=zn, in0=rot, scalar=0.0, in1=b4(mnn4),
                                           op0=A.min, op1=A.mult)
            nc.scalar.activation(zn, zn, AF.Identity, bias=MAGIC)
            qq = qkpool.tile([128, 4, HD], f32r, tag="qq")
            nc.vector.scalar_tensor_tensor(out=qq, in0=zp, scalar=-MAGIC, in1=b4(spal),
                                           op0=A.add, op1=A.mult)
            nc.vector.scalar_tensor_tensor(out=zn, in0=zn, scalar=-MAGIC, in1=b4(snal),
                                           op0=A.add, op1=A.mult)
            nc.vector.tensor_tensor(out=qq, in0=qq, in1=zn, op=A.add)

            if DBG:
                natf = dpool.tile([128, 512], f32, tag="d512")
                nc.vector.tensor_copy(out=natf, in_=qk_ps)
                nc.sync.dma_start(out=dbg_nat[tsl, 0:512], in_=natf)
                vf = dpool.tile([128, 256], f32, tag="d256")
                nc.vector.tensor_copy(out=vf, in_=v_ps)
                nc.sync.dma_start(out=dbg_nat[tsl, 512:768], in_=vf)
                qqf = dpool.tile([128, 512], f32, tag="d512")
                nc.vector.tensor_copy(out=qqf, in_=qq)
                nc.sync.dma_start(out=dbg_qq[tsl, :], in_=qqf)
                nc.sync.dma_start(out=dbg_g[tsl, :], in_=gate_so[:, i, :])
                vaf = dpool.tile([128, 2, 128], f32, tag="d256")
                nc.vector.tensor_copy(out=vaf, in_=vaug[:, :, i, 0:HD])
                nc.sync.dma_start(out=dbg_va[tsl, :], in_=vaf.rearrange("p h e -> p (h e)"))
            # transpose to qkT[:, :, tile]
            psq = psY.tile([128, 4, 128], f32r, tag="yu")
            for c in range(4):
                nc.tensor.transpose(psq[:, c, :], qq[:, c, :], ident)
            nc.scalar.activation(qkT[:, :, tsl], psq, AF.Identity)


        for J in range(NSTRIP):
            for h in range(HLOC):
                emit_strip(h, J)
            emit_proj(J)

    nc.compile()
    return nc


def _quantized_linear_np(x, eps=1e-5):
    # bit-mirror of reference.quantized_linear forward (fp32 op order)
    xn = np.minimum(np.min(x, -1, keepdims=True), np.float32(-eps))
    xp = np.maximum(np.max(x, -1, keepdims=True), np.float32(eps))
    f127 = np.float32(127.0)
    xpq = np.round((x / xp) * f127) / f127 * xp
    xnq = np.round((x / xn) * f127) / f127 * xn
    return np.where(x >= 0, xpq, xnq).astype(np.float32)


def _host_prep(inputs):
    x = np.asarray(inputs["x"], np.float32)
    ve = np.asarray(inputs["ve"], np.float32)
    lam = np.asarray(inputs["sa_lambdas"], np.float32)
    cos = np.asarray(inputs["cos"], np.float32)
    sin = np.asarray(inputs["sin"], np.float32)
    qkvo = np.asarray(inputs["qkvo_w"], np.float32)
    gw = np.asarray(inputs["gate_w"], np.float32)

    # ternary weight codes + global scales (host-exact)
    w3 = qkvo[:3]
    s3 = np.maximum(
        np.mean(np.mean(np.abs(w3), -1, keepdims=True), -2, keepdims=True),
        np.float32(1e-5)).astype(np.float32)            # [3,1,1]
    codes3 = np.clip(np.round(w3 / s3), -1.0, 1.0).astype(np.float32)
    s_qkv = s3.reshape(3)
    s_o = np.float32(max(np.abs(qkvo[3]).mean(dtype=np.float32), np.float32(1e-5)))
    codes_o = np.clip(np.round(qkvo[3] / s_o), -1.0, 1.0).astype(np.float32)

    # x fake-quant (exact) per batch, pre-transposed
    xqT = [np.ascontiguousarray(_quantized_linear_np(x[b]).T) for b in range(B)]

    c2 = np.ascontiguousarray(np.concatenate([cos, cos], 1))   # [T,128]
    s2 = np.ascontiguousarray(np.concatenate([sin, -sin], 1))  # [T,128]

    scal = np.zeros((128, 10), np.float32)
    scal[:, 0] = scal[:, 1] = s_qkv[0]
    scal[:, 2] = scal[:, 3] = s_qkv[1]
    scal[:, 4:8] = scal[:, 0:4] ** 2
    scal[:, 8] = lam[0] * s_qkv[2]
    scal[:, 9] = s_o

    in_maps = []
    for c in range(8):
        b, g = divmod(c, 4)
        rows = slice(g * ELOC, (g + 1) * ELOC)
        wq = np.concatenate([codes3[s][rows].T for s in range(3)], axis=1)  # [1024,768]
        in_maps.append({
            "xqT": xqT[b],
            "tau": np.ascontiguousarray(wq),
            "tauo": np.ascontiguousarray(codes_o.T[rows]),
            "c2": c2,
            "s2": s2,
            "vel": np.ascontiguousarray(lam[1] * ve[b][:, rows]),
            "gw": np.ascontiguousarray(gw[2 * g:2 * g + 2].T),
            "scal": scal,
        })
    return in_maps


def _gather(res):
    outs = [r["outp"] for r in res.results]
    out = np.empty((B, T, DIM), np.float32)
    for b in range(B):
        out[b] = outs[4 * b] + outs[4 * b + 1] + outs[4 * b + 2] + outs[4 * b + 3]
    return out


def kernel(**inputs):
    from concourse.bass_utils import run_bass_kernel_spmd

    if "nc" not in _CACHE:
        _CACHE["nc"] = _build()
    nc = _CACHE["nc"]
    in_maps = _host_prep(inputs)
    res = run_bass_kernel_spmd(nc, in_maps, core_ids=list(range(8)))
    return _gather(res)


if __name__ == "__main__":
    import reference as R
    inputs = R.setup_inputs()
    out = kernel(**{k: np.asarray(v) for k, v in inputs.items()})
    print(out.shape, out.dtype)


# revision 22
# speedup vs baseline: 1.0413x; 1.0413x over previous
"""Trainium2 Bass kernel for nn_CausalSelfAttention (modded-nanogpt quantized attention).

Sharding: 8 cores = 2 batches x 4 head-groups (2 heads each).
Each core computes QKV for its 2 heads from the full x[b], runs causal
attention + gating for those heads, and produces a partial output
projection (its 256 feature columns of w_o); the host sums the 4 partials
per batch.

Host-side prep (exact, bit-mirrors the reference):
 - x int8 fake-quant computed in numpy and shipped PRE-TRANSPOSED (xqT),
   removing the whole on-device x-quant pipeline and its PE transposes.
 - qkv / o weights ternary-quantized on host; only the {-1,0,1} CODES are
   shipped (exact in any dtype); global scales fold into per-head alpha,
   the v-mix scalar and the gate scalar.

Device-side structure per core:
 - All matmul operands are float32r: full PE speed (1 cycle/row when the
   moving free dim >= 256) and EXACT for integer codes; ~1.5e-4 rel err
   on value matmuls (scores / PV / proj) - well within tolerance.
 - QKV natural [t,768] from xqT-codes @ w-codes (exact integer arithmetic).
 - q/k chains processed JOINTLY as [128, 4, 128] (q0 q1 k0 k1): rmsnorm
   alpha (rms folded with the ternary scale), rotary, int8 fake-quant with
   per-head scalars applied through stride-0 broadcast APs.
 - scores transposed S_T[tk,tq] = kT.T @ qT; exp on ACT (scale fused),
   causal diag mask via affine_select; y = E.T @ [v|1] per 512-wide tq
   strip; normalize+gate per token; PE-transpose y into yT; partial
   output projection from yT against w_o codes.
"""

import numpy as np

B, T, DIM, H, HD = 2, 2048, 1024, 8, 128
ATTN_SCALE = 0.12
F32_EPS = float(np.finfo(np.float32).eps)
MAGIC = float(np.float32(1.5 * 2 ** 23))  # RNE round for |x| < 2^22
NT = T // 128          # 16 t-tiles
ND = DIM // 128        # 8 d-tiles
HLOC = 2               # heads per core
ELOC = HLOC * HD       # 256 local features
NSTRIP = T // 512      # 4 tq strips per head

_CACHE = {}


def _build():
    import concourse.bass as bass
    import concourse.mybir as mybir
    import concourse.tile as tile
    from concourse import bacc
    from concourse.masks import make_identity
    from contextlib import ExitStack

    f32 = mybir.dt.float32
    f32r = mybir.dt.float32r
    A = mybir.AluOpType
    AF = mybir.ActivationFunctionType
    X = mybir.AxisListType.X

    nc = bacc.Bacc(trn_type="TRN2")

    # extra activation-bias constants (Bass pre-registers only 0.0/1.0)
    for _v in (MAGIC, F32_EPS):
        _t = nc.alloc_sbuf_tensor(f"const-float32-{_v}", [128, 1], f32)
        nc.gpsimd.memset(_t.ap(), _v)
        nc.const_aps.aps[(f32, _v)] = _t.ap()
    nc.all_engine_barrier()

    xqT_d = nc.dram_tensor("xqT", [DIM, T], f32r, kind="ExternalInput")
    tau_d = nc.dram_tensor("tau", [DIM, 3 * ELOC], f32r, kind="ExternalInput")
    tauo_d = nc.dram_tensor("tauo", [ELOC, DIM], f32r, kind="ExternalInput")
    c2_d = nc.dram_tensor("c2", [T, HD], f32, kind="ExternalInput")
    s2_d = nc.dram_tensor("s2", [T, HD], f32, kind="ExternalInput")
    vel_d = nc.dram_tensor("vel", [T, ELOC], f32, kind="ExternalInput")
    gw_d = nc.dram_tensor("gw", [12, HLOC], f32r, kind="ExternalInput")
    # scal cols: 0:4 s4=[sq,sq,sk,sk], 4:8 sq2=[sq^2,..,sk^2], 8 lam0*sv, 9 s_o
    scal_d = nc.dram_tensor("scal", [128, 10], f32, kind="ExternalInput")
    outp = nc.dram_tensor("outp", [T, DIM], f32, kind="ExternalOutput")
    import os
    DBG = int(os.environ.get("KDBG", "0"))
    if DBG:
        dbg_nat = nc.dram_tensor("dbg_nat", [T, 768], f32, kind="ExternalOutput")
        dbg_qq = nc.dram_tensor("dbg_qq", [T, 512], f32, kind="ExternalOutput")
        dbg_g = nc.dram_tensor("dbg_g", [T, HLOC], f32, kind="ExternalOutput")
        dbg_va = nc.dram_tensor("dbg_va", [T, ELOC], f32, kind="ExternalOutput")
        dbg_yt = nc.dram_tensor("dbg_yt", [T, ELOC], f32, kind="ExternalOutput")

    def bc(src, like):
        b, _ = bass.broadcast_tensor_aps(src, like)
        return b

    with tile.TileContext(nc) as tc, ExitStack() as ctx:
        singles = ctx.enter_context(tc.tile_pool(name="singles", bufs=1))
        inpool = ctx.enter_context(tc.tile_pool(name="inpool", bufs=2))
        qkpool = ctx.enter_context(tc.tile_pool(name="qkpool", bufs=2))
        scl = ctx.enter_context(tc.tile_pool(name="scl", bufs=4))
        epool = ctx.enter_context(tc.tile_pool(name="epool", bufs=2 if DBG else 3))
        ypool = ctx.enter_context(tc.tile_pool(name="ypool", bufs=2))
        opool = ctx.enter_context(tc.tile_pool(name="opool", bufs=2))
        dpool = ctx.enter_context(tc.tile_pool(name="dpool", bufs=1)) if DBG else None
        psQK = ctx.enter_context(tc.tile_pool(name="psQK", bufs=2, space="PSUM"))
        psY = ctx.enter_context(tc.tile_pool(name="psY", bufs=4, space="PSUM"))

        def ts(out, in0, s1, s2=None, op0=A.mult, op1=None, eng=None):
            e = eng if eng is not None else nc.vector
            kw = {}
            if op1 is not None:
                kw["op1"] = op1
            e.tensor_scalar(out=out, in0=in0, scalar1=s1, scalar2=s2, op0=op0, **kw)

        # ---------------- constants / persistent inputs ----------------
        ident_f = singles.tile([128, 128], f32)
        make_identity(nc, ident_f)
        ident = singles.tile([128, 128], f32r)
        nc.vector.tensor_copy(out=ident, in_=ident_f)

        scal_sb = singles.tile([128, 10], f32)
        nc.sync.dma_start(out=scal_sb, in_=scal_d[:, :])
        gw_sb = singles.tile([12, HLOC], f32r)
        nc.sync.dma_start(out=gw_sb, in_=gw_d[:, :])

        tau = singles.tile([128, ND, 3 * ELOC], f32r)
        nc.sync.dma_start(out=tau, in_=tau_d.rearrange("(n p) e -> p n e", p=128))
        tauo = singles.tile([128, HLOC, DIM], f32r)
        nc.sync.dma_start(out=tauo, in_=tauo_d.rearrange("(h p) d -> p h d", p=128))
        xqT = singles.tile([128, ND, T], f32r)
        nc.sync.dma_start(out=xqT, in_=xqT_d.rearrange("(n p) t -> p n t", p=128))

        # persistent activations
        qkT = singles.tile([128, 4, T], f32r)          # q0 q1 k0 k1, [hd, t]
        vaug = singles.tile([128, HLOC, NT, HD + 2], f32r)
        ones_f = singles.tile([128, NT, 2], f32)
        nc.gpsimd.memset(ones_f[:, :, 0:1], 1.0)
        nc.gpsimd.memset(ones_f[:, :, 1:2], 0.0)
        for _h in range(HLOC):
            nc.vector.tensor_copy(out=vaug[:, _h, :, HD:HD + 2], in_=ones_f)
        gate_so = singles.tile([128, NT, HLOC], f32)   # sigmoid(gate)*s_o
        yT = singles.tile([128, HLOC, T], f32r)        # [hd, t] gated y

        # ======== attention strip + projection emitters (interleaved) ========
        def emit_strip(h, J):
            Jsl = slice(J * 512, (J + 1) * 512)
            yu0 = psY.tile([128, HD + 2], f32, tag="yu")
            yu1 = psY.tile([128, HD + 2], f32, tag="yu")
            yu2 = psY.tile([128, HD + 2], f32, tag="yu")
            yu3 = psY.tile([128, HD + 2], f32, tag="yu")
            yu = [yu0, yu1, yu2, yu3]
            for i in range(4 * J + 4):
                st = psQK.tile([128, 4, 128], f32, tag="qk")
                stf = st.rearrange("p a b -> p (a b)")
                nc.tensor.matmul(stf, qkT[:, 2 + h, i * 128:(i + 1) * 128],
                                 qkT[:, h, Jsl], start=True, stop=True)
                lo = max(0, 128 * (i - 4 * J))
                E = epool.tile([128, 512], f32r, tag="E")
                nc.scalar.activation(E[:, lo:512], stf[:, lo:512], AF.Exp,
                                     scale=ATTN_SCALE)
                if i >= 4 * J:
                    dl = 128 * (i - 4 * J)
                    nc.gpsimd.affine_select(
                        out=E[:, dl:dl + 128], in_=E[:, dl:dl + 128],
                        compare_op=A.is_ge, fill=0.0, base=0,
                        pattern=[[1, 128]], channel_multiplier=-1)
                for j in range(max(4 * J, i), 4 * J + 4):
                    jj = j - 4 * J
                    nc.tensor.matmul(yu[jj], E[:, jj * 128:(jj + 1) * 128],
                                     vaug[:, h, i, :],
                                     start=(i == 0), stop=(i == j),
                                     skip_group_check=True)
            for jj in range(4):
                j = 4 * J + jj
                den = scl.tile([128, 1], f32, tag="den")
                nc.vector.reciprocal(out=den, in_=yu[jj][:, HD:HD + 1])
                gam = scl.tile([128, 1], f32, tag="gam")
                nc.vector.tensor_tensor(out=gam, in0=den,
                                        in1=gate_so[:, j, h:h + 1], op=A.mult)
                ynat = ypool.tile([128, HD], f32r, tag="ynat")
                ts(ynat, yu[jj][:, 0:HD], gam)
                psy = psY.tile([128, 4, 128], f32r, tag="yu")
                nc.tensor.transpose(psy[:, 0, :], ynat, ident)
                nc.vector.tensor_copy(out=yT[:, h, j * 128:(j + 1) * 128],
                                       in_=psy[:, 0, :])

        def emit_proj(J):
            for jj in range(4):
                j = 4 * J + jj
                jsl = slice(j * 128, (j + 1) * 128)
                osb = opool.tile([128, DIM], f32, tag="osb")
                for half in range(2):
                    op_ = psQK.tile([128, 512], f32, tag="qk")
                    for h in range(HLOC):
                        nc.tensor.matmul(op_, yT[:, h, jsl],
                                         tauo[:, h, half * 512:(half + 1) * 512],
                                         start=(h == 0), stop=(h == HLOC - 1))
                    nc.vector.tensor_copy(out=osb[:, half * 512:(half + 1) * 512],
                                          in_=op_)
                nc.sync.dma_start(out=outp[jsl, :], in_=osb)
                if DBG:
                    pd = psY.tile([128, 4, 128], f32r, tag="yu")
                    for h in range(HLOC):
                        nc.tensor.transpose(pd[:, h, :], yT[:, h, jsl], ident)
                    ydf = dpool.tile([128, 256], f32, tag="d256")
                    nc.vector.tensor_copy(out=ydf, in_=pd[:, 0:2, :])
                    nc.sync.dma_start(out=dbg_yt[jsl, :], in_=ydf)

        # ======== phase 1: per t-tile QKV -> q/k chain -> v mix ========
        for i in range(NT):
            tsl = slice(i * 128, (i + 1) * 128)
            c2t = inpool.tile([128, 1, HD], f32, tag="c2t")
            s2t = inpool.tile([128, 1, HD], f32, tag="s2t")
            velt = inpool.tile([128, HLOC, HD], f32, tag="velt")
            nc.sync.dma_start(out=c2t, in_=c2_d[tsl, :].rearrange("p (o e) -> p o e", o=1))
            nc.sync.dma_start(out=s2t, in_=s2_d[tsl, :].rearrange("p (o e) -> p o e", o=1))
            nc.sync.dma_start(out=velt, in_=vel_d[tsl, :].rearrange("p (h e) -> p h e", h=HLOC))

            # QKV matmuls (codes x codes: exact)
            qk_ps = psQK.tile([128, 2 * ELOC], f32, tag="qk")
            v_ps = psQK.tile([128, ELOC], f32, tag="v")
            for d in range(ND):
                nc.tensor.matmul(qk_ps, xqT[:, d, tsl], tau[:, d, 0:2 * ELOC],
                                 start=(d == 0), stop=(d == ND - 1))
                nc.tensor.matmul(v_ps, xqT[:, d, tsl], tau[:, d, 2 * ELOC:3 * ELOC],
                                 start=(d == 0), stop=(d == ND - 1))

            # gate logits from quantized-x rows 0..11
            g_ps = psY.tile([128, HLOC], f32, tag="yu")
            nc.tensor.matmul(g_ps, xqT[0:12, 0, tsl], gw_sb, start=True, stop=True)
            nc.scalar.activation(gate_so[:, i, :], g_ps, AF.Sigmoid)
            ts(gate_so[:, i, :], gate_so[:, i, :], scal_sb[:, 9:10])

            # v mix: vaug = (lam0*sv)*v_hat + vel  (both heads at once)
            v3 = v_ps.rearrange("p (h e) -> p h e", h=HLOC)
            nc.vector.scalar_tensor_tensor(
                out=vaug[:, :, i, 0:HD], in0=v3, scalar=scal_sb[:, 8:9],
                in1=velt, op0=A.mult, op1=A.add)

            # ---- joint q/k chain on [128, 4, 128] ----
            nat = qkpool.tile([128, 4, HD], f32r, tag="nat")
            nc.scalar.activation(nat, qk_ps, AF.Identity)

            t2 = qkpool.tile([128, 4, HD], f32, tag="t2")
            nc.vector.tensor_tensor(out=t2, in0=nat, in1=nat, op=A.mult)
            ssq = scl.tile([128, 4], f32, tag="ssq")
            nc.vector.tensor_reduce(
                out=ssq.rearrange("p (h o) -> p h o", o=1), in_=t2, axis=X, op=A.add)
            # alpha = s / sqrt(ssq*s^2/HD + eps)   [128,4]
            al4 = scl.tile([128, 4], f32, tag="al4")
            nc.vector.scalar_tensor_tensor(out=al4, in0=ssq, scalar=1.0 / HD,
                                           in1=scal_sb[:, 4:8], op0=A.mult, op1=A.mult)
            nc.scalar.activation(al4, al4, AF.Sqrt, bias=F32_EPS)
            nc.vector.reciprocal(out=al4, in_=al4)
            nc.vector.tensor_tensor(out=al4, in0=al4, in1=scal_sb[:, 0:4], op=A.mult)

            # rotary: rot = nat*c2 + shuf(nat)*s2  (c2/s2 broadcast over 4 chunks)
            rot = qkpool.tile([128, 4, HD], f32, tag="rot")
            nc.vector.tensor_tensor(out=rot, in0=nat, in1=bc(c2t, rot), op=A.mult)
            nc.vector.tensor_tensor(out=t2[:, :, 0:64], in0=nat[:, :, 64:128],
                                    in1=bc(s2t[:, :, 0:64], t2[:, :, 0:64]), op=A.mult)
            nc.vector.tensor_tensor(out=t2[:, :, 64:128], in0=nat[:, :, 0:64],
                                    in1=bc(s2t[:, :, 64:128], t2[:, :, 64:128]), op=A.mult)
            nc.vector.tensor_tensor(out=rot, in0=rot, in1=t2, op=A.add)

            # per-head min/max and quant scales
            mx4 = scl.tile([128, 4], f32, tag="mx4")
            mn4 = scl.tile([128, 4], f32, tag="mn4")
            nc.vector.tensor_reduce(
                out=mx4.rearrange("p (h o) -> p h o", o=1), in_=rot, axis=X, op=A.max)
            nc.vector.tensor_reduce(
                out=mn4.rearrange("p (h o) -> p h o", o=1), in_=rot, axis=X, op=A.min)
            xpm = scl.tile([128, 4], f32, tag="xpm")
            xnm = scl.tile([128, 4], f32, tag="xnm")
            ts(xpm, mx4, 1e-5, None, A.max)
            ts(xnm, mn4, -1e-5, None, A.min)
            mp4 = scl.tile([128, 4], f32, tag="mp4")
            mnn4 = scl.tile([128, 4], f32, tag="mnn4")
            nc.vector.reciprocal(out=mp4, in_=xpm)
            nc.vector.reciprocal(out=mnn4, in_=xnm)
            ts(mp4, mp4, 127.0)
            ts(mnn4, mnn4, 127.0)
            spal = scl.tile([128, 4], f32, tag="spal")
            snal = scl.tile([128, 4], f32, tag="snal")
            nc.vector.scalar_tensor_tensor(out=spal, in0=xpm, scalar=1.0 / 127.0,
                                           in1=al4, op0=A.mult, op1=A.mult)
            nc.vector.scalar_tensor_tensor(out=snal, in0=xnm, scalar=1.0 / 127.0,
                                           in1=al4, op0=A.mult, op1=A.mult)

            def b4(t_):  # [128,4] -> broadcast [128,4,128]
                return bc(t_.rearrange("p (h o) -> p h o", o=1), rot)

            # two-branch int8 fake-quant (RNE via magic constant)
            zp = qkpool.tile([128, 4, HD], f32, tag="zp")
            zn = qkpool.tile([128, 4, HD], f32, tag="zn")
            nc.vector.scalar_tensor_tensor(out=zp, in0=rot, scalar=0.0, in1=b4(mp4),
                                           op0=A.max, op1=A.mult)
            nc.scalar.activation(zp, zp, AF.Identity, bias=MAGIC)
            nc.vector.scalar_tensor_tensor(out=zn, in0=rot, scalar=0.0, in1=b4(mnn4),
                                           op0=A.min, op1=A.mult)
            nc.scalar.activation(zn, zn, AF.Identity, bias=MAGIC)
            qq = qkpool.tile([128, 4, HD], f32r, tag="qq")
            nc.vector.scalar_tensor_tensor(out=qq, in0=zp, scalar=-MAGIC, in1=b4(spal),
                                           op0=A.add, op1=A.mult)
            nc.vector.scalar_tensor_tensor(out=zn, in0=zn, scalar=-MAGIC, in1=b4(snal),
                                           op0=A.add, op1=A.mult)
            nc.vector.tensor_tensor(out=qq, in0=qq, in1=zn, op=A.add)

            if DBG:
                natf = dpool.tile([128, 512], f32, tag="d512")
                nc.vector.tensor_copy(out=natf, in_=qk_ps)
                nc.sync.dma_start(out=dbg_nat[tsl, 0:512], in_=natf)
                vf = dpool.tile([128, 256], f32, tag="d256")
                nc.vector.tensor_copy(out=vf, in_=v_ps)
                nc.sync.dma_start(out=dbg_nat[tsl, 512:768], in_=vf)
                qqf = dpool.tile([128, 512], f32, tag="d512")
                nc.vector.tensor_copy(out=qqf, in_=qq)
                nc.sync.dma_start(out=dbg_qq[tsl, :], in_=qqf)
                nc.sync.dma_start(out=dbg_g[tsl, :], in_=gate_so[:, i, :])
                vaf = dpool.tile([128, 2, 128], f32, tag="d256")
                nc.vector.tensor_copy(out=vaf, in_=vaug[:, :, i, 0:HD])
                nc.sync.dma_start(out=dbg_va[tsl, :], in_=vaf.rearrange("p h e -> p (h e)"))
            # transpose to qkT[:, :, tile]
            psq = psY.tile([128, 4, 128], f32r, tag="yu")
            for c in range(4):
                nc.tensor.transpose(psq[:, c, :], qq[:, c, :], ident)
            nc.scalar.activation(qkT[:, :, tsl], psq, AF.Identity)


        for J in range(NSTRIP):
            for h in range(HLOC):
                emit_strip(h, J)
            emit_proj(J)

    nc.compile()
    return nc


def _quantized_linear_np(x, eps=1e-5):
    # bit-mirror of reference.quantized_linear forward (fp32 op order)
    xn = np.minimum(np.min(x, -1, keepdims=True), np.float32(-eps))
    xp = np.maximum(np.max(x, -1, keepdims=True), np.float32(eps))
    f127 = np.float32(127.0)
    xpq = np.round((x / xp) * f127) / f127 * xp
    xnq = np.round((x / xn) * f127) / f127 * xn
    return np.where(x >= 0, xpq, xnq).astype(np.float32)


def _host_prep(inputs):
    x = np.asarray(inputs["x"], np.float32)
    ve = np.asarray(inputs["ve"], np.float32)
    lam = np.asarray(inputs["sa_lambdas"], np.float32)
    cos = np.asarray(inputs["cos"], np.float32)
    sin = np.asarray(inputs["sin"], np.float32)
    qkvo = np.asarray(inputs["qkvo_w"], np.float32)
    gw = np.asarray(inputs["gate_w"], np.float32)

    # ternary weight codes + global scales (host-exact)
    w3 = qkvo[:3]
    s3 = np.maximum(
        np.mean(np.mean(np.abs(w3), -1, keepdims=True), -2, keepdims=True),
        np.float32(1e-5)).astype(np.float32)            # [3,1,1]
    codes3 = np.clip(np.round(w3 / s3), -1.0, 1.0).astype(np.float32)
    s_qkv = s3.reshape(3)
    s_o = np.float32(max(np.abs(qkvo[3]).mean(dtype=np.float32), np.float32(1e-5)))
    codes_o = np.clip(np.round(qkvo[3] / s_o), -1.0, 1.0).astype(np.float32)

    # x fake-quant (exact) per batch, pre-transposed
    xqT = [np.ascontiguousarray(_quantized_linear_np(x[b]).T) for b in range(B)]

    c2 = np.ascontiguousarray(np.concatenate([cos, cos], 1))   # [T,128]
    s2 = np.ascontiguousarray(np.concatenate([sin, -sin], 1))  # [T,128]

    scal = np.zeros((128, 10), np.float32)
    scal[:, 0] = scal[:, 1] = s_qkv[0]
    scal[:, 2] = scal[:, 3] = s_qkv[1]
    scal[:, 4:8] = scal[:, 0:4] ** 2
    scal[:, 8] = lam[0] * s_qkv[2]
    scal[:, 9] = s_o

    in_maps = []
    for c in range(8):
        b, g = divmod(c, 4)
        rows = slice(g * ELOC, (g + 1) * ELOC)
        wq = np.concatenate([codes3[s][rows].T for s in range(3)], axis=1)  # [1024,768]
        in_maps.append({
            "xqT": xqT[b],
            "tau": np.ascontiguousarray(wq),
            "tauo": np.ascontiguousarray(codes_o.T[rows]),
            "c2": c2,
            "s2": s2,
            "vel": np.ascontiguousarray(lam[1] * ve[b][:, rows]),
            "gw": np.ascontiguousarray(gw[2 * g:2 * g + 2].T),
            "scal": scal,
        })
    return in_maps


def _gather(res):
    outs = [r["outp"] for r in res.results]
    out = np.empty((B, T, DIM), np.float32)
    for b in range(B):
        out[b] = outs[4 * b] + outs[4 * b + 1] + outs[4 * b + 2] + outs[4 * b + 3]
    return out


def kernel(**inputs):
    from concourse.bass_utils import run_bass_kernel_spmd

    if "nc" not in _CACHE:
        _CACHE["nc"] = _build()
    nc = _CACHE["nc"]
    in_maps = _host_prep(inputs)
    res = run_bass_kernel_spmd(nc, in_maps, core_ids=list(range(8)))
    return _gather(res)


if __name__ == "__main__":
    import reference as R
    inputs = R.setup_inputs()
    out = kernel(**{k: np.asarray(v) for k, v in inputs.items()})
    print(out.shape, out.dtype)
